# revision 1
# baseline (speedup 1.0000x reference)
"""Trainium2 Bass kernel for nn_Net_12421045420310 (GNN edge-conditioned message passing).

Sharding (8 cores):
 - Nodes block-sharded: core c owns nodes [c*3125, (c+1)*3125).
 - Edges assigned to the core owning their dst node, sorted by dst within the
   shard -> scatter-mean is purely core-local (no all-reduce); node state is
   re-replicated with one AllGather per conv step (bf16).
 - edge_index3/edge_attr3 position-sharded 5000/core; final outputs stitched
   on host from each core's ExternalOutput slice.

Device pipeline per conv step (per core):
 - Wedge ([128,64,64] per 128-edge tile) is recomputed on TensorE from the
   SBUF-resident rh1^T (bf16) and a host-permuted f-major W_e2 (bf16); never
   written to HBM.
 - out[src] rows gathered from a bf16 node-state replica via indirect DMA.
 - per-edge GEMV msg[e,f] = sum_d out_src[e,d]*Wedge[e,d,f] on VectorE:
   broadcast-AP tensor_tensor multiply + innermost reduce.
 - scatter-mean via 0/1 selection-matrix matmuls on TensorE (1/cnt is folded
   into a per-edge tensor_scalar multiply), producing agg^T in [feat, node]
   layout, which feeds the GRU matmuls without transposes.
 - GRU gates on PE(+PSUM accumulation)/ScalarE/VectorE in transposed layout;
   h^T transposed back per 128-node chunk on TensorE, DMA'd to DRAM, AllGather.
"""

import math

import numpy as np
import ml_dtypes

import sys
import types

# This axon client build lacks antenv.axon_hooks; stub it so that
# run_bass_kernel_spmd(trace=True) degrades to the no-profile path.
if "antenv.axon_hooks" not in sys.modules:
    try:
        import antenv.axon_hooks  # noqa: F401
    except ImportError:
        _stub = types.ModuleType("antenv.axon_hooks")
        _stub.get_axon_ntff_profile_hook = lambda: None
        sys.modules["antenv.axon_hooks"] = _stub

import concourse.bass as bass
import concourse.bacc as bacc
import concourse.tile as tile
import concourse.mybir as mybir
from concourse.bass_utils import run_bass_kernel_spmd

AF = mybir.ActivationFunctionType
ALU = mybir.AluOpType
DT = mybir.dt

BF16 = DT.bfloat16
F32 = DT.float32
I32 = DT.int32


class Cfg:
    def __init__(self, N=25000, E=50000, E3=40000, DIM=64, NCORES=8, K_SLOTS=3,
                 REPS=1, do_wedge=True, do_apply=True, do_gather=True):
        assert N % NCORES == 0 and E3 % NCORES == 0 and DIM == 64
        self.N, self.E, self.E3, self.DIM, self.NCORES = N, E, E3, DIM, NCORES
        self.NPC = N // NCORES                      # nodes per core
        self.NCH = math.ceil(self.NPC / 128)        # node chunks per core
        self.NPC_PAD = self.NCH * 128
        self.E3PC = E3 // NCORES
        self.NT3 = math.ceil(self.E3PC / 128)
        self.E3_PAD = self.NT3 * 128
        self.K_SLOTS = K_SLOTS
        self.REPS = REPS
        self.do_wedge = do_wedge
        self.do_apply = do_apply
        self.do_gather = do_gather
        # filled by prep():
        self.NT_E = None
        self.EC_PAD = None
        self.has_be2 = False

    def slot_chunk(self, t, slot):
        """Node chunk targeted by scatter slot (t, slot); identical across cores."""
        c = t * self.NCH // self.NT_E + slot - (self.K_SLOTS // 2)
        return min(max(c, 0), self.NCH - 1)


def _remap_node(cfg, n):
    """Map global node id -> row in the interleaved padded replica layout."""
    c = n // cfg.NPC
    l = n % cfg.NPC
    return c * cfg.NPC_PAD + (l % 128) * cfg.NCH + (l // 128)


def prep(cfg, inputs):
    """Host-side sharding/layout. Returns (in_maps, cfg) with cfg.NT_E set."""
    f32 = np.float32
    x = np.asarray(inputs["x"], f32)
    edge_attr = np.asarray(inputs["edge_attr"], f32)
    edge_attr3 = np.asarray(inputs["edge_attr3"], f32)
    ei = np.asarray(inputs["edge_index"]).astype(np.int64)
    ei3 = np.asarray(inputs["edge_index3"]).astype(np.int64)

    W_node = np.asarray(inputs["W_node"], f32); b_node = np.asarray(inputs["b_node"], f32)
    W_ea = np.asarray(inputs["W_ea"], f32); b_ea = np.asarray(inputs["b_ea"], f32)
    W_e1 = np.asarray(inputs["W_e1"], f32); b_e1 = np.asarray(inputs["b_e1"], f32)
    W_e2 = np.asarray(inputs["W_e2"], f32); b_e2 = np.asarray(inputs["b_e2"], f32)
    conv_bias = np.asarray(inputs["conv_bias"], f32)
    W_ih = np.asarray(inputs["W_ih"], f32); b_ih = np.asarray(inputs["b_ih"], f32)
    W_hh = np.asarray(inputs["W_hh"], f32); b_hh = np.asarray(inputs["b_hh"], f32)
    W_l1 = np.asarray(inputs["W_l1"], f32); b_l1 = np.asarray(inputs["b_l1"], f32)
    W_l2 = np.asarray(inputs["W_l2"], f32); b_l2 = np.asarray(inputs["b_l2"], f32)

    D = cfg.DIM
    src, dst = ei[0], ei[1]
    owner = dst // cfg.NPC

    # per-core edge shards sorted by dst
    shards = []
    max_ec = 0
    for c in range(cfg.NCORES):
        sel = np.nonzero(owner == c)[0]
        order = sel[np.argsort(dst[sel], kind="stable")]
        shards.append(order)
        max_ec = max(max_ec, len(order))
    cfg.NT_E = max(1, math.ceil(max_ec / 128))
    cfg.EC_PAD = cfg.NT_E * 128
    cfg.has_be2 = bool(np.abs(b_e2).max() > 0)

    # f-major permutation of W_e2: W_e2p[k, f*64+d] = W_e2[k, d*64+f]
    W_e2p = W_e2.reshape(128, D, D).transpose(0, 2, 1).reshape(128, D * D)
    b_e2p = b_e2.reshape(D, D).T.reshape(1, D * D)

    bf16 = ml_dtypes.bfloat16
    common = {
        "W_node": W_node,
        "bnode": b_node[:, None].copy(),
        "W_ea": W_ea,
        "bea": b_ea[:, None].copy(),
        "W_e1": W_e1,
        "be1": b_e1[:, None].copy(),
        "W_e2p": W_e2p.astype(bf16),
        "be2p": b_e2p.astype(bf16),
        "cbias": conv_bias[:, None].copy(),
        "Wih_rzT": W_ih[0:2 * D].T.copy(),
        "Wih_nT": W_ih[2 * D:3 * D].T.copy(),
        "Whh_rzT": W_hh[0:2 * D].T.copy(),
        "Whh_nT": W_hh[2 * D:3 * D].T.copy(),
        "br": (b_ih[0:D] + b_hh[0:D])[:, None].copy(),
        "bz": (b_ih[D:2 * D] + b_hh[D:2 * D])[:, None].copy(),
        "bin_": b_ih[2 * D:3 * D][:, None].copy(),
        "bhn": b_hh[2 * D:3 * D][:, None].copy(),
        "Wl1a": (0.5 * W_l1[0:D]).astype(bf16),
        "Wl1b": W_l1[D:].copy(),
        "bl1": b_l1[:, None].copy(),
        "Wl2": W_l2,
        "bl2": b_l2[:, None].copy(),
        "id128": np.eye(128, dtype=f32),
        "ones1": np.ones((1, 128), bf16),
    }

    in_maps = []
    for c in range(cfg.NCORES):
        order = shards[c]
        ec = len(order)
        e_src = src[order]
        e_dst_loc = dst[order] - c * cfg.NPC
        cnt = np.bincount(e_dst_loc, minlength=cfg.NPC).astype(f32)
        cnt = np.maximum(cnt, 1.0)

        # padded per-edge arrays
        gsrc = np.zeros(cfg.EC_PAD, np.int32)
        gsrc[:ec] = np.array([_remap_node(cfg, int(n)) for n in e_src], np.int32)
        assert cfg.NCORES * cfg.NPC_PAD < 2**15, "dma_gather needs int16 indices"
        cntinv = np.zeros(cfg.EC_PAD, f32)
        cntinv[:ec] = 1.0 / cnt[e_dst_loc]
        dl_pad = np.full(cfg.EC_PAD, 2**30, np.int64)
        dl_pad[:ec] = e_dst_loc

        eaT = np.zeros((edge_attr.shape[1], cfg.EC_PAD), f32)
        eaT[:, :ec] = edge_attr[order].T

        # scatter selection blocks: NS = K_SLOTS*NT_E blocks of [128,128], 0/1
        S_all = np.zeros((128, cfg.K_SLOTS * cfg.NT_E * 128), bf16)
        for t in range(cfg.NT_E):
            seg = dl_pad[t * 128:(t + 1) * 128]
            chunks = {cfg.slot_chunk(t, s): s for s in range(cfg.K_SLOTS)}
            for r in range(128):
                d_ = seg[r]
                if d_ >= cfg.NPC:
                    continue
                c2 = int(d_) // 128
                if c2 not in chunks:
                    raise RuntimeError(
                        f"scatter slot coverage failure core={c} t={t} c2={c2}; "
                        f"increase K_SLOTS"
                    )
                s_idx = t * cfg.K_SLOTS + chunks[c2]
                S_all[r, s_idx * 128 + (int(d_) - c2 * 128)] = 1.0

        # node features (transposed, padded, local slice); column l = local node l
        xT = np.zeros((x.shape[1], cfg.NPC_PAD), f32)
        xT[:, :cfg.NPC] = x[c * cfg.NPC:(c + 1) * cfg.NPC].T

        # readout shard
        sl3 = slice(c * cfg.E3PC, (c + 1) * cfg.E3PC)
        ga3 = np.zeros(cfg.E3_PAD, np.int32)
        gb3 = np.zeros(cfg.E3_PAD, np.int32)
        ga3[:cfg.E3PC] = [_remap_node(cfg, int(n)) for n in ei3[0, sl3]]
        gb3[:cfg.E3PC] = [_remap_node(cfg, int(n)) for n in ei3[1, sl3]]
        ea3T = np.zeros((edge_attr3.shape[1], cfg.E3_PAD), f32)
        ea3T[:, :cfg.E3PC] = edge_attr3[sl3].T

        m = dict(common)
        m.update({
            "xT": xT,
            "eaT": eaT,
            "S_all": S_all,
            "gsrc": gsrc.reshape(cfg.NT_E, 128).T.copy(),
            "cntinv": cntinv.reshape(cfg.NT_E, 128).T.copy(),
            "ga3": ga3.reshape(cfg.NT3, 128).T.copy(),
            "gb3": gb3.reshape(cfg.NT3, 128).T.copy(),
            "ea3T": ea3T,
        })
        in_maps.append(m)
    return in_maps, cfg


def _blocks(total, width):
    out = []
    o = 0
    while o < total:
        w = min(width, total - o)
        out.append((o, w))
        o += w
    return out


def build_program(cfg):
    D = cfg.DIM
    NC = cfg.NCORES
    nc = bacc.Bacc("TRN2", target_bir_lowering=False, debug=False, num_devices=NC)

    def din(name, shape, dt=F32):
        return nc.dram_tensor(name, shape, dt, kind="ExternalInput").ap()

    # ---- I/O declarations ----
    xT = din("xT", [8, cfg.NPC_PAD])
    eaT_in = din("eaT", [19, cfg.EC_PAD])
    S_in = din("S_all", [128, cfg.K_SLOTS * cfg.NT_E * 128], BF16)
    gsrc_in = din("gsrc", [128, cfg.NT_E], I32)
    cntinv_in = din("cntinv", [128, cfg.NT_E])
    ga3_in = din("ga3", [128, cfg.NT3], I32)
    gb3_in = din("gb3", [128, cfg.NT3], I32)
    ea3T_in = din("ea3T", [8, cfg.E3_PAD])

    W_node = din("W_node", [8, D]); bnode = din("bnode", [D, 1])
    W_ea = din("W_ea", [19, 12]); bea = din("bea", [12, 1])
    W_e1 = din("W_e1", [12, 128]); be1 = din("be1", [128, 1])
    W_e2p = din("W_e2p", [128, D * D], BF16)
    be2p = din("be2p", [1, D * D], BF16)
    ones1 = din("ones1", [1, 128], BF16)
    cbias = din("cbias", [D, 1])
    Wih_rzT = din("Wih_rzT", [D, 2 * D]); Whh_rzT = din("Whh_rzT", [D, 2 * D])
    Wih_nT = din("Wih_nT", [D, D]); Whh_nT = din("Whh_nT", [D, D])
    br = din("br", [D, 1]); bz = din("bz", [D, 1]); bin_ = din("bin_", [D, 1]); bhn = din("bhn", [D, 1])
    Wl1a = din("Wl1a", [D, 128], BF16); Wl1b = din("Wl1b", [8, 128]); bl1 = din("bl1", [128, 1])
    Wl2 = din("Wl2", [128, 1]); bl2 = din("bl2", [1, 1])
    id128 = din("id128", [128, 128])

    y_out = nc.dram_tensor("y", [1, cfg.E3_PAD], F32, kind="ExternalOutput").ap()

    NREP = NC * cfg.NPC_PAD

    with tile.TileContext(nc) as tc:
        # ---- DRAM internals ----
        h_loc = []
        h_rep = []
        for k in range(4):
            h_loc.append(tc.tile([cfg.NPC_PAD, D], BF16, space="DRAM",
                                 name=f"h_loc{k}")[0])
            h_rep.append(tc.tile([NREP, D], BF16, space="DRAM",
                                 addr_space="Shared", name=f"h_rep{k}")[0])

        # ---- persistent SBUF ----
        pers_cm = tc.tile_pool(name="pers", bufs=1)
        pers_p = pers_cm.__enter__()

        def load(name, ap_in, dt=None):
            t = pers_p.tile(list(ap_in.shape), dt or ap_in.dtype, name=name, tag=name)
            nc.sync.dma_start(out=t[:], in_=ap_in[:])
            return t

        S_sb = load("S_sb", S_in)
        gsrc_sb = load("gsrc_sb", gsrc_in)
        cntinv_sb = load("cntinv_sb", cntinv_in)
        ga3_sb = load("ga3_sb", ga3_in)
        gb3_sb = load("gb3_sb", gb3_in)
        W_node_sb = load("W_node_sb", W_node); bnode_sb = load("bnode_sb", bnode)
        W_ea_sb = load("W_ea_sb", W_ea); bea_sb = load("bea_sb", bea)
        W_e1_sb = load("W_e1_sb", W_e1); be1_sb = load("be1_sb", be1)
        W_e2p_sb = load("W_e2p_sb", W_e2p)
        be2p_sb = load("be2p_sb", be2p) if cfg.has_be2 else None
        ones1_sb = load("ones1_sb", ones1) if cfg.has_be2 else None
        cbias_sb = load("cbias_sb", cbias)
        Wih_rzT_sb = load("Wih_rzT_sb", Wih_rzT)
        Whh_rzT_sb = load("Whh_rzT_sb", Whh_rzT)
        Wih_nT_sb = load("Wih_nT_sb", Wih_nT)
        Whh_nT_sb = load("Whh_nT_sb", Whh_nT)
        br_sb = load("br_sb", br); bz_sb = load("bz_sb", bz)
        bin_sb = load("bin_sb", bin_); bhn_sb = load("bhn_sb", bhn)
        Wl1a_sb = load("Wl1a_sb", Wl1a); Wl1b_sb = load("Wl1b_sb", Wl1b); bl1_sb = load("bl1_sb", bl1)
        Wl2_sb = load("Wl2_sb", Wl2); bl2_sb = load("bl2_sb", bl2)
        id_sb = load("id_sb", id128)

        rh1T_sb = pers_p.tile([128, cfg.EC_PAD], BF16, name="rh1T_sb", tag="rh1T_sb")
        hTs = [pers_p.tile([D, cfg.NPC_PAD], F32, name=f"hT{k}", tag=f"hT{k}") for k in range(2)]
        hT = [hTs[0], hTs[1], hTs[0], hTs[1]]
        mT_sb = pers_p.tile([D, cfg.NPC_PAD], F32, name="mT_sb", tag="mT_sb")
        msg_sb = pers_p.tile([128, cfg.NT_E * D], BF16, name="msg_sb", tag="msg_sb")
        pairT_sb = pers_p.tile([D, cfg.E3_PAD], BF16, name="pairT_sb", tag="pairT_sb")
        hrow_sb = pers_p.tile([128, cfg.NCH * D], BF16, name="hrow_sb", tag="hrow_sb")

        # ---- pools ----
        with (
            tc.tile_pool(name="wpsum", bufs=2, space="PSUM") as wpsum_p,
            tc.tile_pool(name="sp", bufs=4, space="PSUM") as sp_p,
            tc.tile_pool(name="work", bufs=2) as work_p,
            tc.tile_pool(name="gath", bufs=4) as gath_p,
            tc.tile_pool(name="gruw", bufs=2) as gru_p,
            tc.tile_pool(name="strw", bufs=2) as str_p,
        ):
            # ablation constants
            osrc_const = pers_p.tile([128, D], BF16, name="osrc_c", tag="osrc_c")
            wedge_const = pers_p.tile([128, D * D], BF16, name="wedge_c", tag="wedge_c")
            if not cfg.do_gather:
                nc.vector.memset(osrc_const[:], 0)
            if not cfg.do_wedge:
                nc.vector.memset(wedge_const[:], 0)
            if not cfg.do_apply:
                nc.vector.memset(msg_sb[:], 0)

            for _rep in range(cfg.REPS):
                # ---- edge MLP (once): rh1T = relu(W_e1^T @ relu(W_ea^T @ ea^T)) ----
                for (o, w) in _blocks(cfg.EC_PAD, 256):
                    eat_in = str_p.tile([19, 256], F32, tag="eat_in")
                    nc.sync.dma_start(out=eat_in[:, :w], in_=eaT_in[:, o:o + w])
                    ps = sp_p.tile([128, 512], F32, tag="sp")
                    nc.tensor.matmul(out=ps[:12, :w], lhsT=W_ea_sb[:], rhs=eat_in[:, :w],
                                     start=True, stop=True)
                    eat = str_p.tile([12, 256], F32, tag="eat")
                    nc.scalar.activation(out=eat[:, :w], in_=ps[:12, :w],
                                         func=AF.Relu, bias=bea_sb[:])
                    ps2 = sp_p.tile([128, 512], F32, tag="sp")
                    nc.tensor.matmul(out=ps2[:, :w], lhsT=W_e1_sb[:], rhs=eat[:, :w],
                                     start=True, stop=True)
                    nc.scalar.activation(out=rh1T_sb[:, o:o + w], in_=ps2[:, :w],
                                         func=AF.Relu, bias=be1_sb[:])

                # ---- node MLP: h0^T = relu(W_node^T @ x^T) ----
                for (o, w) in _blocks(cfg.NPC_PAD, 256):
                    xt_in = str_p.tile([8, 256], F32, tag="xt_in")
                    nc.sync.dma_start(out=xt_in[:, :w], in_=xT[:, o:o + w])
                    ps = sp_p.tile([128, 512], F32, tag="sp")
                    nc.tensor.matmul(out=ps[:D, :w], lhsT=W_node_sb[:], rhs=xt_in[:, :w],
                                     start=True, stop=True)
                    nc.scalar.activation(out=hT[0][:, o:o + w], in_=ps[:D, :w],
                                         func=AF.Relu, bias=bnode_sb[:])

            # ---- helper: transpose hT -> rows, DMA, AllGather ----
                def publish(k):
                    for c2 in range(cfg.NCH):
                        tp = sp_p.tile([128, 512], F32, tag="sp")
                        nc.tensor.transpose(out=tp[:, :D], in_=hT[k][:, c2 * 128:(c2 + 1) * 128],
                                            identity=id_sb[:D, :D])
                        nc.vector.tensor_copy(out=hrow_sb[:, c2 * D:(c2 + 1) * D], in_=tp[:, :D])
                    nc.sync.dma_start(
                        out=h_loc[k][:].rearrange("(p c) d -> p (c d)", p=128),
                        in_=hrow_sb[:],
                    )
                    nc.gpsimd.collective_compute(
                        "AllGather", ALU.bypass,
                        replica_groups=[list(range(NC))],
                        ins=[h_loc[k][:].opt()],
                        outs=[h_rep[k][:].opt()],
                    )

                publish(0)

                # ---- conv steps ----
                for k in range(3):
                    # msg phase
                    for t in range(cfg.NT_E):
                        if True:
                            if cfg.do_gather:
                                osrc = gath_p.tile([128, D], BF16, tag="osrc")
                                nc.gpsimd.indirect_dma_start(
                                    out=osrc[:], out_offset=None,
                                    in_=h_rep[k][:],
                                    in_offset=bass.IndirectOffsetOnAxis(
                                        ap=gsrc_sb[:, t:t + 1], axis=0),
                                )
                            else:
                                osrc = osrc_const
                            if not cfg.do_wedge:
                                wedge = wedge_const
                            if cfg.do_wedge:
                                wedge = work_p.tile([128, D * D], BF16, tag="wedge")
                                for jh in range(4):
                                    wp = wpsum_p.tile([128, 1024], F32, tag="wp")
                                    for q in range(2):
                                        j = jh * 2 + q
                                        nc.tensor.matmul(
                                            out=wp[:, q * 512:(q + 1) * 512],
                                            lhsT=rh1T_sb[:, t * 128:(t + 1) * 128],
                                            rhs=W_e2p_sb[:, j * 512:(j + 1) * 512],
                                            start=True, stop=not cfg.has_be2)
                                        if cfg.has_be2:
                                            nc.tensor.matmul(
                                                out=wp[:, q * 512:(q + 1) * 512],
                                                lhsT=ones1_sb[:],
                                                rhs=be2p_sb[:, j * 512:(j + 1) * 512],
                                                start=False, stop=True)
                                    nc.scalar.activation(
                                        out=wedge[:, jh * 1024:(jh + 1) * 1024],
                                        in_=wp[:], func=AF.Copy)
                            if not cfg.do_apply:
                                continue
                            tmp = work_p.tile([128, D * D], BF16, tag="tmp")
                            tmp3 = tmp[:].rearrange("p (f d) -> p f d", d=D)
                            nc.vector.tensor_tensor(
                                out=tmp3,
                                in0=osrc[:].unsqueeze(1).to_broadcast([128, D, D]),
                                in1=wedge[:].rearrange("p (f d) -> p f d", d=D),
                                op=ALU.mult)
                            for dd in (D // 2, D // 4, D // 8):
                                nc.vector.tensor_add(
                                    out=tmp3[:, :, 0:dd], in0=tmp3[:, :, 0:dd],
                                    in1=tmp3[:, :, dd:2 * dd])
                            msgf = work_p.tile([128, D], F32, tag="msgf")
                            nc.vector.tensor_reduce(
                                out=msgf[:], in_=tmp3[:, :, 0:D // 8],
                                axis=mybir.AxisListType.X, op=ALU.add)
                            nc.vector.tensor_scalar_mul(
                                out=msg_sb[:, t * D:(t + 1) * D], in0=msgf[:],
                                scalar1=cntinv_sb[:, t:t + 1])

                    # scatter phase: aggT per node chunk
                    contributors = [[] for _ in range(cfg.NCH)]
                    for t in range(cfg.NT_E):
                        for s in range(cfg.K_SLOTS):
                            contributors[cfg.slot_chunk(t, s)].append((t, t * cfg.K_SLOTS + s))
                    for c2 in range(cfg.NCH):
                        conts = contributors[c2]
                        ap_ = sp_p.tile([128, 512], F32, tag="sp")
                        for j, (t, sidx) in enumerate(conts):
                            nc.tensor.matmul(
                                out=ap_[:D, :128], lhsT=msg_sb[:, t * D:(t + 1) * D],
                                rhs=S_sb[:, sidx * 128:(sidx + 1) * 128],
                                start=(j == 0), stop=(j == len(conts) - 1))
                        nc.scalar.activation(
                            out=mT_sb[:, c2 * 128:(c2 + 1) * 128], in_=ap_[:D, :128],
                            func=AF.Relu, bias=cbias_sb[:])

                    # GRU phase
                    for (o, w) in _blocks(cfg.NPC_PAD, 512):
                        rp = sp_p.tile([128, 512], F32, tag="sp")
                        nc.tensor.matmul(out=rp[:D, :w], lhsT=Wih_rzT_sb[:, :D],
                                         rhs=mT_sb[:, o:o + w], start=True, stop=False)
                        nc.tensor.matmul(out=rp[:D, :w], lhsT=Whh_rzT_sb[:, :D],
                                         rhs=hT[k][:, o:o + w], start=False, stop=True)
                        rt = gru_p.tile([D, 512], F32, tag="rt")
                        nc.scalar.activation(out=rt[:, :w], in_=rp[:D, :w],
                                             func=AF.Sigmoid, bias=br_sb[:])
                        zp = sp_p.tile([128, 512], F32, tag="sp")
                        nc.tensor.matmul(out=zp[:D, :w], lhsT=Wih_rzT_sb[:, D:],
                                         rhs=mT_sb[:, o:o + w], start=True, stop=False)
                        nc.tensor.matmul(out=zp[:D, :w], lhsT=Whh_rzT_sb[:, D:],
                                         rhs=hT[k][:, o:o + w], start=False, stop=True)
                        zt = gru_p.tile([D, 512], F32, tag="zt")
                        nc.scalar.activation(out=zt[:, :w], in_=zp[:D, :w],
                                             func=AF.Sigmoid, bias=bz_sb[:])
                        np_ = sp_p.tile([128, 512], F32, tag="sp")
                        nc.tensor.matmul(out=np_[:D, :w], lhsT=Wih_nT_sb[:],
                                         rhs=mT_sb[:, o:o + w], start=True, stop=True)
                        hnp = sp_p.tile([128, 512], F32, tag="sp")
                        nc.tensor.matmul(out=hnp[:D, :w], lhsT=Whh_nT_sb[:],
                                         rhs=hT[k][:, o:o + w], start=True, stop=True)
                        hnb = gru_p.tile([D, 512], F32, tag="hnb")
                        nc.scalar.activation(out=hnb[:, :w], in_=hnp[:D, :w],
                                             func=AF.Identity, bias=bhn_sb[:])
                        nc.vector.tensor_mul(out=hnb[:, :w], in0=rt[:, :w], in1=hnb[:, :w])
                        nc.vector.tensor_add(out=hnb[:, :w], in0=np_[:D, :w], in1=hnb[:, :w])
                        ng = gru_p.tile([D, 512], F32, tag="ng")
                        nc.scalar.activation(out=ng[:, :w], in_=hnb[:, :w],
                                             func=AF.Tanh, bias=bin_sb[:])
                        hmn = gru_p.tile([D, 512], F32, tag="hmn")
                        nc.vector.tensor_sub(out=hmn[:, :w], in0=hT[k][:, o:o + w], in1=ng[:, :w])
                        nc.vector.tensor_mul(out=hmn[:, :w], in0=zt[:, :w], in1=hmn[:, :w])
                        nc.vector.tensor_add(out=hT[k + 1][:, o:o + w], in0=ng[:, :w],
                                             in1=hmn[:, :w])

                    publish(k + 1)

                # ---- readout ----
                for t3 in range(cfg.NT3):
                    if True:
                        pa = gath_p.tile([128, D], BF16, tag="pa")
                        nc.gpsimd.indirect_dma_start(
                            out=pa[:], out_offset=None, in_=h_rep[3][:],
                            in_offset=bass.IndirectOffsetOnAxis(
                                ap=ga3_sb[:, t3:t3 + 1], axis=0))
                        pb = gath_p.tile([128, D], BF16, tag="pb")
                        nc.gpsimd.indirect_dma_start(
                            out=pb[:], out_offset=None, in_=h_rep[3][:],
                            in_offset=bass.IndirectOffsetOnAxis(
                                ap=gb3_sb[:, t3:t3 + 1], axis=0))
                        pab = gath_p.tile([128, D], F32, tag="pab")
                        nc.vector.tensor_add(out=pab[:], in0=pa[:], in1=pb[:])
                        tp = sp_p.tile([128, 512], F32, tag="sp")
                        nc.tensor.transpose(out=tp[:D, :128], in_=pab[:], identity=id_sb[:])
                        nc.vector.tensor_copy(out=pairT_sb[:, t3 * 128:(t3 + 1) * 128], in_=tp[:D, :128])

                for (o, w) in _blocks(cfg.E3_PAD, 256):
                    ea3t = str_p.tile([8, 256], F32, tag="ea3t")
                    nc.sync.dma_start(out=ea3t[:, :w], in_=ea3T_in[:, o:o + w])
                    y1p = sp_p.tile([128, 512], F32, tag="sp")
                    nc.tensor.matmul(out=y1p[:, :w], lhsT=Wl1a_sb[:],
                                     rhs=pairT_sb[:, o:o + w], start=True, stop=False)
                    nc.tensor.matmul(out=y1p[:, :w], lhsT=Wl1b_sb[:],
                                     rhs=ea3t[:, :w], start=False, stop=True)
                    y1 = str_p.tile([128, 256], F32, tag="y1")
                    nc.scalar.activation(out=y1[:, :w], in_=y1p[:, :w],
                                         func=AF.Relu, bias=bl1_sb[:])
                    yp = sp_p.tile([128, 512], F32, tag="sp")
                    nc.tensor.matmul(out=yp[:1, :w], lhsT=Wl2_sb[:], rhs=y1[:, :w],
                                     start=True, stop=True)
                    yb = str_p.tile([1, 256], F32, tag="yb")
                    nc.scalar.activation(out=yb[:, :w], in_=yp[:1, :w],
                                         func=AF.Identity, bias=bl2_sb[:])
                    nc.sync.dma_start(out=y_out[:, o:o + w], in_=yb[:, :w])


        pers_cm.__exit__(None, None, None)

    nc.compile()
    return nc


_CACHE = {}


def run(inputs, cfg=None, trace=False):
    cfg = cfg or Cfg()
    in_maps, cfg = prep(cfg, inputs)
    key = (cfg.N, cfg.E, cfg.E3, cfg.NT_E, cfg.has_be2, cfg.REPS,
           cfg.do_wedge, cfg.do_apply, cfg.do_gather)
    if key not in _CACHE:
        _CACHE[key] = build_program(cfg)
    nc = _CACHE[key]
    res = run_bass_kernel_spmd(nc, in_maps, core_ids=list(range(cfg.NCORES)),
                               trace=trace)
    ys = [res.results[c]["y"][0, :cfg.E3PC] for c in range(cfg.NCORES)]
    out = np.concatenate(ys).astype(np.float32)
    return out, res


def kernel(**inputs) -> np.ndarray:
    out, _ = run(inputs)
    return out


def _pjrt_callable(nc, in_maps):
    """Build a cached jitted shard_map callable mirroring bass2jax's tail."""
    import jax
    import jax.numpy as jnp
    from jax.sharding import Mesh, PartitionSpec
    from jax.experimental.shard_map import shard_map
    from concourse import bass2jax
    import concourse.mybir as mb

    bass2jax.install_neuronx_cc_hook()
    n_cores = len(in_maps)
    partition_name = nc.partition_id_tensor.name if nc.partition_id_tensor else None
    in_names, out_names, out_avals, zero_outs = [], [], [], []
    for alloc in nc.m.functions[0].allocations:
        if not isinstance(alloc, mb.MemoryLocationSet):
            continue
        name = alloc.memorylocations[0].name
        if alloc.kind == "ExternalInput":
            if name != partition_name:
                in_names.append(name)
        elif alloc.kind == "ExternalOutput":
            out_names.append(name)
            shape = tuple(alloc.tensor_shape)
            dtype = mb.dt.np(alloc.dtype)
            out_avals.append(jax.core.ShapedArray(shape, dtype))
            zero_outs.append(np.zeros(shape, dtype))
    n_params = len(in_names)
    n_outs = len(out_avals)
    in_names_full = list(in_names) + out_names
    if partition_name is not None:
        in_names_full.append(partition_name)
    donate = tuple(range(n_params, n_params + n_outs))

    def _body(*args):
        operands = list(args)
        if partition_name is not None:
            operands.append(bass2jax.partition_id_tensor())
        outs = bass2jax._bass_exec_p.bind(
            *operands,
            out_avals=tuple(out_avals),
            in_names=tuple(in_names_full),
            out_names=tuple(out_names),
            lowering_input_output_aliases=(),
            sim_require_finite=True,
            sim_require_nnan=True,
            nc=nc,
        )
        return tuple(outs)

    devices = jax.devices()[:n_cores]
    mesh = Mesh(np.array(devices), ("core",))
    in_specs = (PartitionSpec("core"),) * (n_params + n_outs)
    out_specs = (PartitionSpec("core"),) * len(out_names)
    sharded = jax.jit(
        shard_map(_body, mesh=mesh, in_specs=in_specs, out_specs=out_specs,
                  check_rep=False),
        donate_argnums=donate, keep_unused=True)
    concat_in = [np.concatenate([np.asarray(in_maps[c][nm]) for c in range(n_cores)],
                                axis=0) for nm in in_names]
    concat_zeros = [np.zeros((n_cores * z.shape[0], *z.shape[1:]), z.dtype)
                    for z in zero_outs]
    return sharded, concat_in, concat_zeros, out_names, out_avals


def timed_run(inputs, cfg=None, repeats=10):
    """Run with steady-state wall timing of the jitted executable."""
    import time as _time
    import jax

    cfg = cfg or Cfg()
    in_maps, cfg = prep(cfg, inputs)
    key = (cfg.N, cfg.E, cfg.E3, cfg.NT_E, cfg.has_be2, cfg.REPS,
           cfg.do_wedge, cfg.do_apply, cfg.do_gather)
    if key not in _CACHE:
        _CACHE[key] = build_program(cfg)
    nc = _CACHE[key]
    sharded, concat_in, concat_zeros, out_names, out_avals = _pjrt_callable(nc, in_maps)
    dev_in = [jax.device_put(a) for a in concat_in]

    times = []
    outs = None
    for i in range(repeats + 1):
        zeros = [jax.device_put(z) for z in concat_zeros]
        for z in zeros:
            z.block_until_ready()
        t0 = _time.perf_counter()
        outs = sharded(*dev_in, *zeros)
        for o in outs:
            o.block_until_ready()
        dt = _time.perf_counter() - t0
        if i > 0:  # skip compile/warmup call
            times.append(dt)
    n_cores = cfg.NCORES
    res = [
        {name: np.asarray(outs[i]).reshape(n_cores, *out_avals[i].shape)[c]
         for i, name in enumerate(out_names)}
        for c in range(n_cores)
    ]
    ys = [res[c]["y"][0, :cfg.E3PC] for c in range(n_cores)]
    out = np.concatenate(ys).astype(np.float32)
    return out, times


def noop_baseline(repeats=10):
    """Wall-time of a trivial 8-core program, to subtract dispatch overhead."""
    import time as _time
    import jax

    nc = bacc.Bacc("TRN2", target_bir_lowering=False, debug=False, num_devices=8)
    a_in = nc.dram_tensor("a", [128, 64], F32, kind="ExternalInput").ap()
    b_out = nc.dram_tensor("b", [128, 64], F32, kind="ExternalOutput").ap()
    with tile.TileContext(nc) as tc:
        t, _f = tc.tile([128, 64], F32, name="t")
        nc.sync.dma_start(out=t[:], in_=a_in[:])
        nc.sync.dma_start(out=b_out[:], in_=t[:])
    nc.compile()
    in_maps = [{"a": np.zeros((128, 64), np.float32)} for _ in range(8)]
    sharded, concat_in, concat_zeros, out_names, out_avals = _pjrt_callable(nc, in_maps)
    dev_in = [jax.device_put(a) for a in concat_in]
    times = []
    for i in range(repeats + 1):
        zeros = [jax.device_put(z) for z in concat_zeros]
        for z in zeros:
            z.block_until_ready()
        t0 = _time.perf_counter()
        outs = sharded(*dev_in, *zeros)
        for o in outs:
            o.block_until_ready()
        dt = _time.perf_counter() - t0
        if i > 0:
            times.append(dt)
    return times



# revision 3
# speedup vs baseline: 35.0460x; 35.0460x over previous
"""Trainium2 Bass kernel for nn_Net_12421045420310 (GNN edge-conditioned message passing).

Sharding (8 cores):
 - Nodes block-sharded: core c owns nodes [c*3125, (c+1)*3125).
 - Edges assigned to the core owning their dst node, sorted by dst within the
   shard -> scatter-mean is purely core-local (no all-reduce); node state is
   re-replicated with one AllGather per conv step (bf16).
 - edge_index3/edge_attr3 position-sharded 5000/core; final outputs stitched
   on host from each core's ExternalOutput slice.

Device pipeline per conv step (per core):
 - Wedge ([128,64,64] per 128-edge tile) is recomputed on TensorE from the
   SBUF-resident rh1^T (bf16) and a host-permuted f-major W_e2 (bf16); never
   written to HBM.
 - out[src] rows gathered from a bf16 node-state replica via indirect DMA.
 - per-edge GEMV msg[e,f] = sum_d out_src[e,d]*Wedge[e,d,f] on VectorE:
   broadcast-AP tensor_tensor multiply + innermost reduce.
 - scatter-mean via 0/1 selection-matrix matmuls on TensorE (1/cnt is folded
   into a per-edge tensor_scalar multiply), producing agg^T in [feat, node]
   layout, which feeds the GRU matmuls without transposes.
 - GRU gates on PE(+PSUM accumulation)/ScalarE/VectorE in transposed layout;
   h^T transposed back per 128-node chunk on TensorE, DMA'd to DRAM, AllGather.
"""

import math

import numpy as np
import ml_dtypes

import sys
import types

# This axon client build lacks antenv.axon_hooks; stub it so that
# run_bass_kernel_spmd(trace=True) degrades to the no-profile path.
if "antenv.axon_hooks" not in sys.modules:
    try:
        import antenv.axon_hooks  # noqa: F401
    except ImportError:
        _stub = types.ModuleType("antenv.axon_hooks")
        _stub.get_axon_ntff_profile_hook = lambda: None
        sys.modules["antenv.axon_hooks"] = _stub

import concourse.bass as bass
import concourse.bacc as bacc
import concourse.tile as tile
import concourse.mybir as mybir
from concourse.bass_utils import run_bass_kernel_spmd

AF = mybir.ActivationFunctionType
ALU = mybir.AluOpType
DT = mybir.dt

BF16 = DT.bfloat16
F32 = DT.float32
I32 = DT.int32


class Cfg:
    def __init__(self, N=25000, E=50000, E3=40000, DIM=64, NCORES=8, K_SLOTS=3,
                 REPS=1, do_wedge=True, do_apply=True, do_gather=True):
        assert N % NCORES == 0 and E3 % NCORES == 0 and DIM == 64
        self.N, self.E, self.E3, self.DIM, self.NCORES = N, E, E3, DIM, NCORES
        self.NPC = N // NCORES                      # nodes per core
        self.NCH = math.ceil(self.NPC / 128)        # node chunks per core
        self.NPC_PAD = self.NCH * 128
        self.E3PC = E3 // NCORES
        self.NT3 = math.ceil(self.E3PC / 128)
        self.E3_PAD = self.NT3 * 128
        self.K_SLOTS = K_SLOTS
        self.REPS = REPS
        self.do_wedge = do_wedge
        self.do_apply = do_apply
        self.do_gather = do_gather
        # filled by prep():
        self.NT_E = None
        self.EC_PAD = None
        self.has_be2 = False

    def slot_chunk(self, t, slot):
        """Node chunk targeted by scatter slot (t, slot); identical across cores."""
        c = t * self.NCH // self.NT_E + slot - (self.K_SLOTS // 2)
        return min(max(c, 0), self.NCH - 1)


def _remap_node(cfg, n):
    """Map global node id -> row in the interleaved padded replica layout."""
    c = n // cfg.NPC
    l = n % cfg.NPC
    return c * cfg.NPC_PAD + (l % 128) * cfg.NCH + (l // 128)


def prep(cfg, inputs):
    """Host-side sharding/layout. Returns (in_maps, cfg) with cfg.NT_E set."""
    f32 = np.float32
    x = np.asarray(inputs["x"], f32)
    edge_attr = np.asarray(inputs["edge_attr"], f32)
    edge_attr3 = np.asarray(inputs["edge_attr3"], f32)
    ei = np.asarray(inputs["edge_index"]).astype(np.int64)
    ei3 = np.asarray(inputs["edge_index3"]).astype(np.int64)

    W_node = np.asarray(inputs["W_node"], f32); b_node = np.asarray(inputs["b_node"], f32)
    W_ea = np.asarray(inputs["W_ea"], f32); b_ea = np.asarray(inputs["b_ea"], f32)
    W_e1 = np.asarray(inputs["W_e1"], f32); b_e1 = np.asarray(inputs["b_e1"], f32)
    W_e2 = np.asarray(inputs["W_e2"], f32); b_e2 = np.asarray(inputs["b_e2"], f32)
    conv_bias = np.asarray(inputs["conv_bias"], f32)
    W_ih = np.asarray(inputs["W_ih"], f32); b_ih = np.asarray(inputs["b_ih"], f32)
    W_hh = np.asarray(inputs["W_hh"], f32); b_hh = np.asarray(inputs["b_hh"], f32)
    W_l1 = np.asarray(inputs["W_l1"], f32); b_l1 = np.asarray(inputs["b_l1"], f32)
    W_l2 = np.asarray(inputs["W_l2"], f32); b_l2 = np.asarray(inputs["b_l2"], f32)

    D = cfg.DIM
    src, dst = ei[0], ei[1]
    owner = dst // cfg.NPC

    # per-core edge shards sorted by dst
    shards = []
    max_ec = 0
    for c in range(cfg.NCORES):
        sel = np.nonzero(owner == c)[0]
        order = sel[np.argsort(dst[sel], kind="stable")]
        shards.append(order)
        max_ec = max(max_ec, len(order))
    cfg.NT_E = max(1, math.ceil(max_ec / 128))
    cfg.EC_PAD = cfg.NT_E * 128
    cfg.has_be2 = bool(np.abs(b_e2).max() > 0)

    # f-major permutation of W_e2: W_e2p[k, f*64+d] = W_e2[k, d*64+f]
    W_e2p = W_e2.reshape(128, D, D).transpose(0, 2, 1).reshape(128, D * D)
    b_e2p = b_e2.reshape(D, D).T.reshape(1, D * D)

    bf16 = ml_dtypes.bfloat16
    common = {
        "W_node": W_node,
        "bnode": b_node[:, None].copy(),
        "W_ea": W_ea,
        "bea": b_ea[:, None].copy(),
        "W_e1": W_e1,
        "be1": b_e1[:, None].copy(),
        "W_e2p": W_e2p.astype(bf16),
        "be2p": b_e2p.astype(bf16),
        "cbias": conv_bias[:, None].copy(),
        "Wih_rzT": W_ih[0:2 * D].T.copy(),
        "Wih_nT": W_ih[2 * D:3 * D].T.copy(),
        "Whh_rzT": W_hh[0:2 * D].T.copy(),
        "Whh_nT": W_hh[2 * D:3 * D].T.copy(),
        "br": (b_ih[0:D] + b_hh[0:D])[:, None].copy(),
        "bz": (b_ih[D:2 * D] + b_hh[D:2 * D])[:, None].copy(),
        "bin_": b_ih[2 * D:3 * D][:, None].copy(),
        "bhn": b_hh[2 * D:3 * D][:, None].copy(),
        "Wl1a": (0.5 * W_l1[0:D]).astype(bf16),
        "Wl1b": W_l1[D:].copy(),
        "bl1": b_l1[:, None].copy(),
        "Wl2": W_l2,
        "bl2": b_l2[:, None].copy(),
        "id128": np.eye(128, dtype=f32),
        "ones1": np.ones((1, 128), bf16),
    }

    in_maps = []
    for c in range(cfg.NCORES):
        order = shards[c]
        ec = len(order)
        e_src = src[order]
        e_dst_loc = dst[order] - c * cfg.NPC
        cnt = np.bincount(e_dst_loc, minlength=cfg.NPC).astype(f32)
        cnt = np.maximum(cnt, 1.0)

        # padded per-edge arrays
        gsrc = np.zeros(cfg.EC_PAD, np.int32)
        gsrc[:ec] = np.array([_remap_node(cfg, int(n)) for n in e_src], np.int32)
        assert cfg.NCORES * cfg.NPC_PAD < 2**15, "dma_gather needs int16 indices"
        cntinv = np.zeros(cfg.EC_PAD, f32)
        cntinv[:ec] = 1.0 / cnt[e_dst_loc]
        dl_pad = np.full(cfg.EC_PAD, 2**30, np.int64)
        dl_pad[:ec] = e_dst_loc

        eaT = np.zeros((edge_attr.shape[1], cfg.EC_PAD), f32)
        eaT[:, :ec] = edge_attr[order].T

        # scatter selection blocks: NS = K_SLOTS*NT_E blocks of [128,128], 0/1
        S_all = np.zeros((128, cfg.K_SLOTS * cfg.NT_E * 128), bf16)
        for t in range(cfg.NT_E):
            seg = dl_pad[t * 128:(t + 1) * 128]
            chunks = {cfg.slot_chunk(t, s): s for s in range(cfg.K_SLOTS)}
            for r in range(128):
                d_ = seg[r]
                if d_ >= cfg.NPC:
                    continue
                c2 = int(d_) // 128
                if c2 not in chunks:
                    raise RuntimeError(
                        f"scatter slot coverage failure core={c} t={t} c2={c2}; "
                        f"increase K_SLOTS"
                    )
                s_idx = t * cfg.K_SLOTS + chunks[c2]
                S_all[r, s_idx * 128 + (int(d_) - c2 * 128)] = 1.0

        # node features (transposed, padded, local slice); column l = local node l
        xT = np.zeros((x.shape[1], cfg.NPC_PAD), f32)
        xT[:, :cfg.NPC] = x[c * cfg.NPC:(c + 1) * cfg.NPC].T

        # readout shard
        sl3 = slice(c * cfg.E3PC, (c + 1) * cfg.E3PC)
        ga3 = np.zeros(cfg.E3_PAD, np.int32)
        gb3 = np.zeros(cfg.E3_PAD, np.int32)
        ga3[:cfg.E3PC] = [_remap_node(cfg, int(n)) for n in ei3[0, sl3]]
        gb3[:cfg.E3PC] = [_remap_node(cfg, int(n)) for n in ei3[1, sl3]]
        ea3T = np.zeros((edge_attr3.shape[1], cfg.E3_PAD), f32)
        ea3T[:, :cfg.E3PC] = edge_attr3[sl3].T

        m = dict(common)
        m.update({
            "xT": xT,
            "eaT": eaT,
            "S_all": S_all,
            "gsrc": gsrc.reshape(cfg.NT_E, 128).T.copy(),
            "cntinv": cntinv.reshape(cfg.NT_E, 128).T.copy(),
            "ga3": ga3.reshape(cfg.NT3, 128).T.copy(),
            "gb3": gb3.reshape(cfg.NT3, 128).T.copy(),
            "ea3T": ea3T,
        })
        in_maps.append(m)
    return in_maps, cfg


def _blocks(total, width):
    out = []
    o = 0
    while o < total:
        w = min(width, total - o)
        out.append((o, w))
        o += w
    return out


def build_program(cfg, sim1=False):
    D = cfg.DIM
    NC = cfg.NCORES
    nc = bacc.Bacc("TRN2", target_bir_lowering=False, debug=False,
                   num_devices=1 if sim1 else NC)

    def din(name, shape, dt=F32):
        return nc.dram_tensor(name, shape, dt, kind="ExternalInput").ap()

    # ---- I/O declarations ----
    xT = din("xT", [8, cfg.NPC_PAD])
    eaT_in = din("eaT", [19, cfg.EC_PAD])
    S_in = din("S_all", [128, cfg.K_SLOTS * cfg.NT_E * 128], BF16)
    gsrc_in = din("gsrc", [128, cfg.NT_E], I32)
    cntinv_in = din("cntinv", [128, cfg.NT_E])
    ga3_in = din("ga3", [128, cfg.NT3], I32)
    gb3_in = din("gb3", [128, cfg.NT3], I32)
    ea3T_in = din("ea3T", [8, cfg.E3_PAD])

    W_node = din("W_node", [8, D]); bnode = din("bnode", [D, 1])
    W_ea = din("W_ea", [19, 12]); bea = din("bea", [12, 1])
    W_e1 = din("W_e1", [12, 128]); be1 = din("be1", [128, 1])
    W_e2p = din("W_e2p", [128, D * D], BF16)
    be2p = din("be2p", [1, D * D], BF16)
    ones1 = din("ones1", [1, 128], BF16)
    cbias = din("cbias", [D, 1])
    Wih_rzT = din("Wih_rzT", [D, 2 * D]); Whh_rzT = din("Whh_rzT", [D, 2 * D])
    Wih_nT = din("Wih_nT", [D, D]); Whh_nT = din("Whh_nT", [D, D])
    br = din("br", [D, 1]); bz = din("bz", [D, 1]); bin_ = din("bin_", [D, 1]); bhn = din("bhn", [D, 1])
    Wl1a = din("Wl1a", [D, 128], BF16); Wl1b = din("Wl1b", [8, 128]); bl1 = din("bl1", [128, 1])
    Wl2 = din("Wl2", [128, 1]); bl2 = din("bl2", [1, 1])
    id128 = din("id128", [128, 128])

    y_out = nc.dram_tensor("y", [1, cfg.E3_PAD], F32, kind="ExternalOutput").ap()

    NREP = NC * cfg.NPC_PAD

    with tile.TileContext(nc) as tc:
        # ---- DRAM internals ----
        h_loc = []
        h_rep = []
        for k in range(4):
            h_loc.append(tc.tile([cfg.NPC_PAD, D], BF16, space="DRAM",
                                 name=f"h_loc{k}")[0])
            h_rep.append(tc.tile([NREP, D], BF16, space="DRAM",
                                 addr_space="Shared", name=f"h_rep{k}")[0])

        # ---- persistent SBUF ----
        pers_cm = tc.tile_pool(name="pers", bufs=1)
        pers_p = pers_cm.__enter__()

        def load(name, ap_in, dt=None):
            t = pers_p.tile(list(ap_in.shape), dt or ap_in.dtype, name=name, tag=name)
            nc.sync.dma_start(out=t[:], in_=ap_in[:])
            return t

        S_sb = load("S_sb", S_in)
        gsrc_sb = load("gsrc_sb", gsrc_in)
        cntinv_sb = load("cntinv_sb", cntinv_in)
        ga3_sb = load("ga3_sb", ga3_in)
        gb3_sb = load("gb3_sb", gb3_in)
        W_node_sb = load("W_node_sb", W_node); bnode_sb = load("bnode_sb", bnode)
        W_ea_sb = load("W_ea_sb", W_ea); bea_sb = load("bea_sb", bea)
        W_e1_sb = load("W_e1_sb", W_e1); be1_sb = load("be1_sb", be1)
        W_e2p_sb = load("W_e2p_sb", W_e2p)
        be2p_sb = load("be2p_sb", be2p) if cfg.has_be2 else None
        ones1_sb = load("ones1_sb", ones1) if cfg.has_be2 else None
        cbias_sb = load("cbias_sb", cbias)
        Wih_rzT_sb = load("Wih_rzT_sb", Wih_rzT)
        Whh_rzT_sb = load("Whh_rzT_sb", Whh_rzT)
        Wih_nT_sb = load("Wih_nT_sb", Wih_nT)
        Whh_nT_sb = load("Whh_nT_sb", Whh_nT)
        br_sb = load("br_sb", br); bz_sb = load("bz_sb", bz)
        bin_sb = load("bin_sb", bin_); bhn_sb = load("bhn_sb", bhn)
        Wl1a_sb = load("Wl1a_sb", Wl1a); Wl1b_sb = load("Wl1b_sb", Wl1b); bl1_sb = load("bl1_sb", bl1)
        Wl2_sb = load("Wl2_sb", Wl2); bl2_sb = load("bl2_sb", bl2)
        id_sb = load("id_sb", id128)

        rh1T_sb = pers_p.tile([128, cfg.EC_PAD], BF16, name="rh1T_sb", tag="rh1T_sb")
        hTs = [pers_p.tile([D, cfg.NPC_PAD], F32, name=f"hT{k}", tag=f"hT{k}") for k in range(2)]
        hT = [hTs[0], hTs[1], hTs[0], hTs[1]]
        mT_sb = pers_p.tile([D, cfg.NPC_PAD], F32, name="mT_sb", tag="mT_sb")
        msg_sb = pers_p.tile([128, cfg.NT_E * D], BF16, name="msg_sb", tag="msg_sb")
        pairT_sb = pers_p.tile([D, cfg.E3_PAD], BF16, name="pairT_sb", tag="pairT_sb")
        hrow_sb = pers_p.tile([128, cfg.NCH * D], BF16, name="hrow_sb", tag="hrow_sb")

        # ---- pools ----
        with (
            tc.tile_pool(name="wpsum", bufs=2, space="PSUM") as wpsum_p,
            tc.tile_pool(name="sp", bufs=4, space="PSUM") as sp_p,
            tc.tile_pool(name="work", bufs=2) as work_p,
            tc.tile_pool(name="gath", bufs=4) as gath_p,
            tc.tile_pool(name="gruw", bufs=2) as gru_p,
            tc.tile_pool(name="strw", bufs=2) as str_p,
        ):
            # ablation constants
            osrc_const = pers_p.tile([128, D], BF16, name="osrc_c", tag="osrc_c")
            wedge_const = pers_p.tile([128, D * D], BF16, name="wedge_c", tag="wedge_c")
            if not cfg.do_gather:
                nc.vector.memset(osrc_const[:], 0)
            if not cfg.do_wedge:
                nc.vector.memset(wedge_const[:], 0)
            if not cfg.do_apply:
                nc.vector.memset(msg_sb[:], 0)

            for _rep in range(cfg.REPS):
                # ---- edge MLP (once): rh1T = relu(W_e1^T @ relu(W_ea^T @ ea^T)) ----
                for (o, w) in _blocks(cfg.EC_PAD, 256):
                    eat_in = str_p.tile([19, 256], F32, tag="eat_in")
                    nc.sync.dma_start(out=eat_in[:, :w], in_=eaT_in[:, o:o + w])
                    ps = sp_p.tile([128, 512], F32, tag="sp")
                    nc.tensor.matmul(out=ps[:12, :w], lhsT=W_ea_sb[:], rhs=eat_in[:, :w],
                                     start=True, stop=True)
                    eat = str_p.tile([12, 256], F32, tag="eat")
                    nc.scalar.activation(out=eat[:, :w], in_=ps[:12, :w],
                                         func=AF.Relu, bias=bea_sb[:])
                    ps2 = sp_p.tile([128, 512], F32, tag="sp")
                    nc.tensor.matmul(out=ps2[:, :w], lhsT=W_e1_sb[:], rhs=eat[:, :w],
                                     start=True, stop=True)
                    nc.scalar.activation(out=rh1T_sb[:, o:o + w], in_=ps2[:, :w],
                                         func=AF.Relu, bias=be1_sb[:])

                # ---- node MLP: h0^T = relu(W_node^T @ x^T) ----
                for (o, w) in _blocks(cfg.NPC_PAD, 256):
                    xt_in = str_p.tile([8, 256], F32, tag="xt_in")
                    nc.sync.dma_start(out=xt_in[:, :w], in_=xT[:, o:o + w])
                    ps = sp_p.tile([128, 512], F32, tag="sp")
                    nc.tensor.matmul(out=ps[:D, :w], lhsT=W_node_sb[:], rhs=xt_in[:, :w],
                                     start=True, stop=True)
                    nc.scalar.activation(out=hT[0][:, o:o + w], in_=ps[:D, :w],
                                         func=AF.Relu, bias=bnode_sb[:])

            # ---- helper: transpose hT -> rows, DMA, AllGather ----
                def publish(k):
                    for c2 in range(cfg.NCH):
                        tp = sp_p.tile([128, 512], F32, tag="sp")
                        nc.tensor.transpose(out=tp[:, :D], in_=hT[k][:, c2 * 128:(c2 + 1) * 128],
                                            identity=id_sb[:D, :D])
                        nc.vector.tensor_copy(out=hrow_sb[:, c2 * D:(c2 + 1) * D], in_=tp[:, :D])
                    nc.sync.dma_start(
                        out=h_loc[k][:].rearrange("(p c) d -> p (c d)", p=128),
                        in_=hrow_sb[:],
                    )
                    if sim1:
                        # cost-model stand-in for AllGather: write own shard
                        nc.sync.dma_start(
                            out=h_rep[k][0:cfg.NPC_PAD, :], in_=h_loc[k][:])
                    else:
                        nc.gpsimd.collective_compute(
                            "AllGather", ALU.bypass,
                            replica_groups=[list(range(NC))],
                            ins=[h_loc[k][:].opt()],
                            outs=[h_rep[k][:].opt()],
                        )

                publish(0)

                # ---- conv steps ----
                for k in range(3):
                    # msg phase
                    for t in range(cfg.NT_E):
                        if True:
                            if cfg.do_gather:
                                osrc = gath_p.tile([128, D], BF16, tag="osrc")
                                nc.gpsimd.indirect_dma_start(
                                    out=osrc[:], out_offset=None,
                                    in_=h_rep[k][:],
                                    in_offset=bass.IndirectOffsetOnAxis(
                                        ap=gsrc_sb[:, t:t + 1], axis=0),
                                )
                            else:
                                osrc = osrc_const
                            if not cfg.do_wedge:
                                wedge = wedge_const
                            if cfg.do_wedge:
                                wedge = work_p.tile([128, D * D], BF16, tag="wedge")
                                for jh in range(4):
                                    wp = wpsum_p.tile([128, 1024], F32, tag="wp")
                                    for q in range(2):
                                        j = jh * 2 + q
                                        nc.tensor.matmul(
                                            out=wp[:, q * 512:(q + 1) * 512],
                                            lhsT=rh1T_sb[:, t * 128:(t + 1) * 128],
                                            rhs=W_e2p_sb[:, j * 512:(j + 1) * 512],
                                            start=True, stop=not cfg.has_be2)
                                        if cfg.has_be2:
                                            nc.tensor.matmul(
                                                out=wp[:, q * 512:(q + 1) * 512],
                                                lhsT=ones1_sb[:],
                                                rhs=be2p_sb[:, j * 512:(j + 1) * 512],
                                                start=False, stop=True)
                                    nc.scalar.activation(
                                        out=wedge[:, jh * 1024:(jh + 1) * 1024],
                                        in_=wp[:], func=AF.Copy)
                            if not cfg.do_apply:
                                continue
                            tmp = work_p.tile([128, D * D], BF16, tag="tmp")
                            tmp3 = tmp[:].rearrange("p (f d) -> p f d", d=D)
                            nc.vector.tensor_tensor(
                                out=tmp3,
                                in0=osrc[:].unsqueeze(1).to_broadcast([128, D, D]),
                                in1=wedge[:].rearrange("p (f d) -> p f d", d=D),
                                op=ALU.mult)
                            for dd in (D // 2, D // 4, D // 8):
                                nc.vector.tensor_add(
                                    out=tmp3[:, :, 0:dd], in0=tmp3[:, :, 0:dd],
                                    in1=tmp3[:, :, dd:2 * dd])
                            msgf = work_p.tile([128, D], F32, tag="msgf")
                            nc.vector.tensor_reduce(
                                out=msgf[:], in_=tmp3[:, :, 0:D // 8],
                                axis=mybir.AxisListType.X, op=ALU.add)
                            nc.vector.tensor_scalar_mul(
                                out=msg_sb[:, t * D:(t + 1) * D], in0=msgf[:],
                                scalar1=cntinv_sb[:, t:t + 1])

                    # scatter phase: aggT per node chunk
                    contributors = [[] for _ in range(cfg.NCH)]
                    for t in range(cfg.NT_E):
                        for s in range(cfg.K_SLOTS):
                            contributors[cfg.slot_chunk(t, s)].append((t, t * cfg.K_SLOTS + s))
                    for c2 in range(cfg.NCH):
                        conts = contributors[c2]
                        ap_ = sp_p.tile([128, 512], F32, tag="sp")
                        for j, (t, sidx) in enumerate(conts):
                            nc.tensor.matmul(
                                out=ap_[:D, :128], lhsT=msg_sb[:, t * D:(t + 1) * D],
                                rhs=S_sb[:, sidx * 128:(sidx + 1) * 128],
                                start=(j == 0), stop=(j == len(conts) - 1))
                        nc.scalar.activation(
                            out=mT_sb[:, c2 * 128:(c2 + 1) * 128], in_=ap_[:D, :128],
                            func=AF.Relu, bias=cbias_sb[:])

                    # GRU phase
                    for (o, w) in _blocks(cfg.NPC_PAD, 512):
                        rp = sp_p.tile([128, 512], F32, tag="sp")
                        nc.tensor.matmul(out=rp[:D, :w], lhsT=Wih_rzT_sb[:, :D],
                                         rhs=mT_sb[:, o:o + w], start=True, stop=False)
                        nc.tensor.matmul(out=rp[:D, :w], lhsT=Whh_rzT_sb[:, :D],
                                         rhs=hT[k][:, o:o + w], start=False, stop=True)
                        rt = gru_p.tile([D, 512], F32, tag="rt")
                        nc.scalar.activation(out=rt[:, :w], in_=rp[:D, :w],
                                             func=AF.Sigmoid, bias=br_sb[:])
                        zp = sp_p.tile([128, 512], F32, tag="sp")
                        nc.tensor.matmul(out=zp[:D, :w], lhsT=Wih_rzT_sb[:, D:],
                                         rhs=mT_sb[:, o:o + w], start=True, stop=False)
                        nc.tensor.matmul(out=zp[:D, :w], lhsT=Whh_rzT_sb[:, D:],
                                         rhs=hT[k][:, o:o + w], start=False, stop=True)
                        zt = gru_p.tile([D, 512], F32, tag="zt")
                        nc.scalar.activation(out=zt[:, :w], in_=zp[:D, :w],
                                             func=AF.Sigmoid, bias=bz_sb[:])
                        np_ = sp_p.tile([128, 512], F32, tag="sp")
                        nc.tensor.matmul(out=np_[:D, :w], lhsT=Wih_nT_sb[:],
                                         rhs=mT_sb[:, o:o + w], start=True, stop=True)
                        hnp = sp_p.tile([128, 512], F32, tag="sp")
                        nc.tensor.matmul(out=hnp[:D, :w], lhsT=Whh_nT_sb[:],
                                         rhs=hT[k][:, o:o + w], start=True, stop=True)
                        hnb = gru_p.tile([D, 512], F32, tag="hnb")
                        nc.scalar.activation(out=hnb[:, :w], in_=hnp[:D, :w],
                                             func=AF.Identity, bias=bhn_sb[:])
                        nc.vector.tensor_mul(out=hnb[:, :w], in0=rt[:, :w], in1=hnb[:, :w])
                        nc.vector.tensor_add(out=hnb[:, :w], in0=np_[:D, :w], in1=hnb[:, :w])
                        ng = gru_p.tile([D, 512], F32, tag="ng")
                        nc.scalar.activation(out=ng[:, :w], in_=hnb[:, :w],
                                             func=AF.Tanh, bias=bin_sb[:])
                        hmn = gru_p.tile([D, 512], F32, tag="hmn")
                        nc.vector.tensor_sub(out=hmn[:, :w], in0=hT[k][:, o:o + w], in1=ng[:, :w])
                        nc.vector.tensor_mul(out=hmn[:, :w], in0=zt[:, :w], in1=hmn[:, :w])
                        nc.vector.tensor_add(out=hT[k + 1][:, o:o + w], in0=ng[:, :w],
                                             in1=hmn[:, :w])

                    publish(k + 1)

                # ---- readout ----
                for t3 in range(cfg.NT3):
                    if True:
                        pa = gath_p.tile([128, D], BF16, tag="pa")
                        nc.gpsimd.indirect_dma_start(
                            out=pa[:], out_offset=None, in_=h_rep[3][:],
                            in_offset=bass.IndirectOffsetOnAxis(
                                ap=ga3_sb[:, t3:t3 + 1], axis=0))
                        pb = gath_p.tile([128, D], BF16, tag="pb")
                        nc.gpsimd.indirect_dma_start(
                            out=pb[:], out_offset=None, in_=h_rep[3][:],
                            in_offset=bass.IndirectOffsetOnAxis(
                                ap=gb3_sb[:, t3:t3 + 1], axis=0))
                        pab = gath_p.tile([128, D], F32, tag="pab")
                        nc.vector.tensor_add(out=pab[:], in0=pa[:], in1=pb[:])
                        tp = sp_p.tile([128, 512], F32, tag="sp")
                        nc.tensor.transpose(out=tp[:D, :128], in_=pab[:], identity=id_sb[:])
                        nc.vector.tensor_copy(out=pairT_sb[:, t3 * 128:(t3 + 1) * 128], in_=tp[:D, :128])

                for (o, w) in _blocks(cfg.E3_PAD, 256):
                    ea3t = str_p.tile([8, 256], F32, tag="ea3t")
                    nc.sync.dma_start(out=ea3t[:, :w], in_=ea3T_in[:, o:o + w])
                    y1p = sp_p.tile([128, 512], F32, tag="sp")
                    nc.tensor.matmul(out=y1p[:, :w], lhsT=Wl1a_sb[:],
                                     rhs=pairT_sb[:, o:o + w], start=True, stop=False)
                    nc.tensor.matmul(out=y1p[:, :w], lhsT=Wl1b_sb[:],
                                     rhs=ea3t[:, :w], start=False, stop=True)
                    y1 = str_p.tile([128, 256], F32, tag="y1")
                    nc.scalar.activation(out=y1[:, :w], in_=y1p[:, :w],
                                         func=AF.Relu, bias=bl1_sb[:])
                    yp = sp_p.tile([128, 512], F32, tag="sp")
                    nc.tensor.matmul(out=yp[:1, :w], lhsT=Wl2_sb[:], rhs=y1[:, :w],
                                     start=True, stop=True)
                    yb = str_p.tile([1, 256], F32, tag="yb")
                    nc.scalar.activation(out=yb[:, :w], in_=yp[:1, :w],
                                         func=AF.Identity, bias=bl2_sb[:])
                    nc.sync.dma_start(out=y_out[:, o:o + w], in_=yb[:, :w])


        pers_cm.__exit__(None, None, None)

    nc.compile()
    return nc


_CACHE = {}


def run(inputs, cfg=None, trace=False):
    cfg = cfg or Cfg()
    in_maps, cfg = prep(cfg, inputs)
    key = (cfg.N, cfg.E, cfg.E3, cfg.NT_E, cfg.has_be2, cfg.REPS,
           cfg.do_wedge, cfg.do_apply, cfg.do_gather)
    if key not in _CACHE:
        _CACHE[key] = build_program(cfg)
    nc = _CACHE[key]
    res = run_bass_kernel_spmd(nc, in_maps, core_ids=list(range(cfg.NCORES)),
                               trace=trace)
    ys = [res.results[c]["y"][0, :cfg.E3PC] for c in range(cfg.NCORES)]
    out = np.concatenate(ys).astype(np.float32)
    return out, res


def kernel(**inputs) -> np.ndarray:
    out, _ = run(inputs)
    return out


def _pjrt_callable(nc, in_maps):
    """Build a cached jitted shard_map callable mirroring bass2jax's tail."""
    import jax
    import jax.numpy as jnp
    from jax.sharding import Mesh, PartitionSpec
    from jax.experimental.shard_map import shard_map
    from concourse import bass2jax
    import concourse.mybir as mb

    bass2jax.install_neuronx_cc_hook()
    n_cores = len(in_maps)
    partition_name = nc.partition_id_tensor.name if nc.partition_id_tensor else None
    in_names, out_names, out_avals, zero_outs = [], [], [], []
    for alloc in nc.m.functions[0].allocations:
        if not isinstance(alloc, mb.MemoryLocationSet):
            continue
        name = alloc.memorylocations[0].name
        if alloc.kind == "ExternalInput":
            if name != partition_name:
                in_names.append(name)
        elif alloc.kind == "ExternalOutput":
            out_names.append(name)
            shape = tuple(alloc.tensor_shape)
            dtype = mb.dt.np(alloc.dtype)
            out_avals.append(jax.core.ShapedArray(shape, dtype))
            zero_outs.append(np.zeros(shape, dtype))
    n_params = len(in_names)
    n_outs = len(out_avals)
    in_names_full = list(in_names) + out_names
    if partition_name is not None:
        in_names_full.append(partition_name)
    donate = tuple(range(n_params, n_params + n_outs))

    def _body(*args):
        operands = list(args)
        if partition_name is not None:
            operands.append(bass2jax.partition_id_tensor())
        outs = bass2jax._bass_exec_p.bind(
            *operands,
            out_avals=tuple(out_avals),
            in_names=tuple(in_names_full),
            out_names=tuple(out_names),
            lowering_input_output_aliases=(),
            sim_require_finite=True,
            sim_require_nnan=True,
            nc=nc,
        )
        return tuple(outs)

    devices = jax.devices()[:n_cores]
    mesh = Mesh(np.array(devices), ("core",))
    in_specs = (PartitionSpec("core"),) * (n_params + n_outs)
    out_specs = (PartitionSpec("core"),) * len(out_names)
    sharded = jax.jit(
        shard_map(_body, mesh=mesh, in_specs=in_specs, out_specs=out_specs,
                  check_rep=False),
        donate_argnums=donate, keep_unused=True)
    concat_in = [np.concatenate([np.asarray(in_maps[c][nm]) for c in range(n_cores)],
                                axis=0) for nm in in_names]
    concat_zeros = [np.zeros((n_cores * z.shape[0], *z.shape[1:]), z.dtype)
                    for z in zero_outs]
    return sharded, concat_in, concat_zeros, out_names, out_avals


def timed_run(inputs, cfg=None, repeats=10):
    """Run with steady-state wall timing of the jitted executable."""
    import time as _time
    import jax

    cfg = cfg or Cfg()
    in_maps, cfg = prep(cfg, inputs)
    key = (cfg.N, cfg.E, cfg.E3, cfg.NT_E, cfg.has_be2, cfg.REPS,
           cfg.do_wedge, cfg.do_apply, cfg.do_gather)
    if key not in _CACHE:
        _CACHE[key] = build_program(cfg)
    nc = _CACHE[key]
    sharded, concat_in, concat_zeros, out_names, out_avals = _pjrt_callable(nc, in_maps)
    dev_in = [jax.device_put(a) for a in concat_in]

    times = []
    outs = None
    for i in range(repeats + 1):
        zeros = [jax.device_put(z) for z in concat_zeros]
        for z in zeros:
            z.block_until_ready()
        t0 = _time.perf_counter()
        outs = sharded(*dev_in, *zeros)
        for o in outs:
            o.block_until_ready()
        dt = _time.perf_counter() - t0
        if i > 0:  # skip compile/warmup call
            times.append(dt)
    n_cores = cfg.NCORES
    res = [
        {name: np.asarray(outs[i]).reshape(n_cores, *out_avals[i].shape)[c]
         for i, name in enumerate(out_names)}
        for c in range(n_cores)
    ]
    ys = [res[c]["y"][0, :cfg.E3PC] for c in range(n_cores)]
    out = np.concatenate(ys).astype(np.float32)
    return out, times


def noop_baseline(repeats=10):
    """Wall-time of a trivial 8-core program, to subtract dispatch overhead."""
    import time as _time
    import jax

    nc = bacc.Bacc("TRN2", target_bir_lowering=False, debug=False, num_devices=8)
    a_in = nc.dram_tensor("a", [128, 64], F32, kind="ExternalInput").ap()
    b_out = nc.dram_tensor("b", [128, 64], F32, kind="ExternalOutput").ap()
    with tile.TileContext(nc) as tc:
        t, _f = tc.tile([128, 64], F32, name="t")
        nc.sync.dma_start(out=t[:], in_=a_in[:])
        nc.sync.dma_start(out=b_out[:], in_=t[:])
    nc.compile()
    in_maps = [{"a": np.zeros((128, 64), np.float32)} for _ in range(8)]
    sharded, concat_in, concat_zeros, out_names, out_avals = _pjrt_callable(nc, in_maps)
    dev_in = [jax.device_put(a) for a in concat_in]
    times = []
    for i in range(repeats + 1):
        zeros = [jax.device_put(z) for z in concat_zeros]
        for z in zeros:
            z.block_until_ready()
        t0 = _time.perf_counter()
        outs = sharded(*dev_in, *zeros)
        for o in outs:
            o.block_until_ready()
        dt = _time.perf_counter() - t0
        if i > 0:
            times.append(dt)
    return times

